# revision 12
# baseline (speedup 1.0000x reference)
import sys
from contextlib import ExitStack

sys.path.insert(0, "/opt/trn_rl_repo")

import numpy as np
import ml_dtypes

import concourse.bass as bass
import concourse.bacc as bacc
import concourse.mybir as mybir
import concourse.tile as tile
from concourse.bass_utils import run_bass_kernel_spmd
from concourse.masks import make_identity

B, N, D, H = 4, 4096, 1024, 16
HD = D // H
NCORES = 8
T = (B * N) // NCORES  # 2048 tokens per core
P = 128
NT = T // P            # 16 token tiles per core
KT = D // P            # 8 contraction tiles
E3 = 3 * D

GP_ATTN = 0   # gpsimd offload disabled: SBUF-BW contention nets zero

_CACHE = {}


def _name(t):
    return t.name if hasattr(t, "name") else t.tensor.name


def _build():
    bf = mybir.dt.bfloat16
    f32 = mybir.dt.float32
    X = mybir.AxisListType.X
    nc = bacc.Bacc(None, target_bir_lowering=False)
    names = {}
    with tile.TileContext(nc) as tc:
        with ExitStack() as ctx:
            dram = ctx.enter_context(tc.tile_pool(name="dram", bufs=1, space="DRAM"))
            xT_d = dram.tile([D, T], bf, kind="ExternalInput")
            wq_d = dram.tile([D, E3], bf, kind="ExternalInput")
            wo_d = dram.tile([D, D], bf, kind="ExternalInput")
            out_d = dram.tile([T, D], f32, kind="ExternalOutput")
            names["xT"] = _name(xT_d)
            names["wqkvT"] = _name(wq_d)
            names["woT"] = _name(wo_d)
            names["out"] = _name(out_d)

            consts = ctx.enter_context(tc.tile_pool(name="consts", bufs=1))
            xT_sb = consts.tile([P, KT, T], bf)
            wq_sb = consts.tile([P, KT, E3], bf)
            wo_sb = consts.tile([P, KT, D], bf)
            ident = consts.tile([P, P], bf)
            make_identity(nc, ident)
            # split input DMAs so tile-0 compute starts as soon as its
            # slices land (whole-tensor uploads cost ~47us of head idle)
            nc.sync.dma_start(
                out=xT_sb[:, :, 0:P],
                in_=xT_d[:, 0:P].rearrange("(k p) t -> p k t", p=P),
            )
            for nch in range(E3 // 512):
                esl = bass.ts(nch, 512)
                nc.sync.dma_start(
                    out=wq_sb[:, :, esl],
                    in_=wq_d[:, esl].rearrange("(k p) e -> p k e", p=P),
                )
            for i in range(1, NT):
                tsl = bass.ts(i, P)
                nc.sync.dma_start(
                    out=xT_sb[:, :, tsl],
                    in_=xT_d[:, tsl].rearrange("(k p) t -> p k t", p=P),
                )
            nc.sync.dma_start(out=wo_sb[:], in_=wo_d[:].rearrange("(k p) e -> p k e", p=P))

            pool = ctx.enter_context(tc.tile_pool(name="work", bufs=2))
            qpool = ctx.enter_context(tc.tile_pool(name="qkvp", bufs=3))
            scratch = ctx.enter_context(tc.tile_pool(name="scratch", bufs=1))
            psum1 = ctx.enter_context(tc.tile_pool(name="psum1", bufs=2, space="PSUM"))
            psum2 = ctx.enter_context(tc.tile_pool(name="psum2", bufs=2, space="PSUM"))
            psum3 = ctx.enter_context(tc.tile_pool(name="psum3", bufs=2, space="PSUM"))

            # warm the PE HAM clock-gate while input DMAs stream.
            # Must be real MATMULs: transpose-mode does not count as PE-busy
            # for HAM, so a transpose warmup leaves the first QKV tile cold.
            for _ in range(32):
                wp = psum2.tile([P, P], f32, tag="pt_warm")
                nc.tensor.matmul(wp[:], ident[:], ident[:], start=True, stop=True)

            # stage1(i): qkv proj + score muls + reduces + softmax -> wn
            # stage2(i): attn (DVE+GP) + merge + transpose + out proj
            # Emitted as s1(0), s1(1), s2(0), s1(2), s2(1), ... so DVE always
            # has stage-1 work queued while GPSIMD chews on stage-2 groups.
            stage2_state = {}

            qkv_state = {}

            def stage1a(i):
                tsl = bass.ts(i, P)
                qkv = qpool.tile([P, E3], bf, tag="qkv")
                for nch in range(E3 // 512):
                    ps = psum1.tile([P, 512], f32, tag="mm1")
                    for k in range(KT):
                        nc.tensor.matmul(
                            ps[:],
                            xT_sb[:, k, tsl],
                            wq_sb[:, k, bass.ts(nch, 512)],
                            start=(k == 0),
                            stop=(k == KT - 1),
                        )
                    nc.scalar.copy(qkv[:, bass.ts(nch, 512)], ps[:])
                qkv_state[i] = qkv

            def stage1b(i):
                qkv = qkv_state[i]
                qv3 = qkv[:, 0:D].rearrange("p (h d) -> p h d", d=HD)
                kv3 = qkv[:, D : 2 * D].rearrange("p (g d) -> p g d", d=HD)
                # one 2x-mode mul for all (h,g) products, then 2x pairwise tree
                big = scratch.tile([P, H * H * HD], bf, tag="big")
                t1 = scratch.tile([P, 8192], bf, tag="t1")
                t2 = scratch.tile([P, 4096], bf, tag="t2")
                t3 = scratch.tile([P, 2048], bf, tag="t3")
                t4 = scratch.tile([P, 1024], bf, tag="t4")
                t5 = scratch.tile([P, 512], bf, tag="t5")
                bigS = big[:].rearrange("p (h g d) -> p h g d", h=H, g=H)
                nc.vector.tensor_mul(
                    bigS,
                    qv3[:, :, None, :].broadcast_to((P, H, H, HD)),
                    kv3[:, None, :, :].broadcast_to((P, H, H, HD)),
                )
                T1 = t1[:].rearrange("p (h g d) -> p h g d", h=H, g=H)
                T2 = t2[:].rearrange("p (h g d) -> p h g d", h=H, g=H)
                T3 = t3[:].rearrange("p (h g d) -> p h g d", h=H, g=H)
                T4 = t4[:].rearrange("p (h g d) -> p h g d", h=H, g=H)
                T5 = t5[:].rearrange("p (h g d) -> p h g d", h=H, g=H)
                nc.vector.tensor_add(T1, bigS[:, :, :, 0:32], bigS[:, :, :, 32:64])
                nc.vector.tensor_add(T2, T1[:, :, :, 0:16], T1[:, :, :, 16:32])
                nc.vector.tensor_add(T3, T2[:, :, :, 0:8], T2[:, :, :, 8:16])
                nc.vector.tensor_add(T4, T3[:, :, :, 0:4], T3[:, :, :, 4:8])
                nc.vector.tensor_add(T5, T4[:, :, :, 0:2], T4[:, :, :, 2:4])
                scores = pool.tile([P, H, H], f32, tag="sc")  # [p, h, g]
                nc.vector.tensor_add(
                    scores[:, :, :, None], T5[:, :, :, 0:1], T5[:, :, :, 1:2]
                )

                # softmax over g (innermost of [p, h, g]); no max-subtract
                we = pool.tile([P, H, H], bf, tag="we")  # exp scores [p, h, g]
                den = pool.tile([P, H], f32, tag="den")
                rec = pool.tile([P, H], f32, tag="rec")
                wn = pool.tile([P, H, H], bf, tag="wn")  # normalized w [p, h, g]
                nc.scalar.activation(
                    we[:], scores[:], mybir.ActivationFunctionType.Exp, scale=1.0 / 32.0
                )
                stage2_state[i] = (we, den, rec, wn, big, t1, t2, t3)

            def stage2(i):
                tsl = bass.ts(i, P)
                we, den, rec, wn, big, t1, t2, t3 = stage2_state.pop(i)
                qkv = qkv_state.pop(i)
                nc.vector.reduce_sum(den[:, :, None], we[:], axis=X)
                nc.vector.reciprocal(rec[:], den[:])
                recb = rec[:, :, None].broadcast_to((P, H, H))
                nc.vector.tensor_mul(wn[:], we[:], recb)
                # attn[p, h, d] = sum_g wn[p, h, g] * v[p, d, g]
                # (host pre-permuted the V weight block so v lands as [d, g])
                vv = qkv[:, 2 * D : 3 * D].rearrange("p (d g) -> p d g", g=H)
                bigA = big[:].rearrange("p (h d g) -> p h d g", h=H, d=HD)
                nc.vector.tensor_mul(
                    bigA,
                    wn[:, :, None, :].broadcast_to((P, H, HD, H)),
                    vv[:, None, :, :].broadcast_to((P, H, HD, H)),
                )
                A1 = t1[:].rearrange("p (h d g) -> p h d g", h=H, d=HD)
                A2 = t2[:].rearrange("p (h d g) -> p h d g", h=H, d=HD)
                A3 = t3[:].rearrange("p (h d g) -> p h d g", h=H, d=HD)
                attn_dve = pool.tile([P, H, HD], bf, tag="attn_dve")
                nc.vector.tensor_add(A1, bigA[:, :, :, 0:8], bigA[:, :, :, 8:16])
                nc.vector.tensor_add(A2, A1[:, :, :, 0:4], A1[:, :, :, 4:8])
                # write the 2-wide level with g outermost in memory so the
                # final add reads two contiguous [P, 1024] halves (2x mode)
                A3g = t3[:].rearrange("p (g h d) -> p h d g", g=2, h=H)
                nc.vector.tensor_add(A3g, A2[:, :, :, 0:2], A2[:, :, :, 2:4])
                nc.vector.tensor_add(
                    attn_dve[:].rearrange("p h d -> p (h d)"),
                    t3[:, 0:1024],
                    t3[:, 1024:2048],
                )

                # transpose attn tile -> [e, t] blocks for output proj
                attnb_flat = attn_dve.rearrange("p h d -> p (h d)")
                attnT = pool.tile([P, KT, P], bf, tag="attnT")
                for c in range(KT):
                    pt = psum2.tile([P, P], bf, tag="pt")
                    nc.tensor.transpose(pt[:], attnb_flat[:, bass.ts(c, P)], ident[:])
                    nc.scalar.copy(attnT[:, c, :], pt[:])

                outt = pool.tile([P, D], f32, tag="outt")
                for nch in range(D // 512):
                    po = psum3.tile([P, 512], f32, tag="po")
                    for k in range(KT):
                        nc.tensor.matmul(
                            po[:],
                            attnT[:, k, :],
                            wo_sb[:, k, bass.ts(nch, 512)],
                            start=(k == 0),
                            stop=(k == KT - 1),
                        )
                    nc.scalar.copy(outt[:, bass.ts(nch, 512)], po[:])
                nc.sync.dma_start(out=out_d[tsl, :], in_=outt[:])

            stage1a(0)
            stage1a(1)
            stage1b(0)
            for i in range(0, NT - 2):
                stage1a(i + 2)
                stage1b(i + 1)
                stage2(i)
            stage1b(NT - 1)
            stage2(NT - 2)
            stage2(NT - 1)
    nc.compile()
    return nc, names


def kernel(x, Wqkv, Wo, bo, trace=False):
    if "nc" not in _CACHE:
        _CACHE["nc"], _CACHE["names"] = _build()
    nc, names = _CACHE["nc"], _CACHE["names"]
    bf = ml_dtypes.bfloat16
    xt = np.ascontiguousarray(
        np.asarray(x, dtype=np.float32).reshape(B * N, D).T
    )  # [D, B*N]
    wqkvT_f = np.ascontiguousarray(np.asarray(Wqkv, dtype=np.float32).T)  # [D, 3D]
    vblk = wqkvT_f[:, 2 * D :].reshape(D, H, HD)
    wqkvT_f[:, 2 * D :] = np.ascontiguousarray(vblk.transpose(0, 2, 1)).reshape(D, D)
    wqkvT = wqkvT_f.astype(bf)
    woT = np.ascontiguousarray(np.asarray(Wo, dtype=np.float32).T).astype(bf)
    in_maps = []
    for c in range(NCORES):
        shard = np.ascontiguousarray(xt[:, c * T : (c + 1) * T]).astype(bf)
        in_maps.append(
            {names["xT"]: shard, names["wqkvT"]: wqkvT, names["woT"]: woT}
        )
    res = run_bass_kernel_spmd(
        nc, in_maps, core_ids=list(range(NCORES)), trace=trace
    )
    shards = [res.results[c][names["out"]] for c in range(NCORES)]
    out = np.concatenate(shards, axis=0).reshape(B, N, D).astype(np.float32)
    out = out + np.asarray(bo, dtype=np.float32)[None, None, :]
    if trace:
        return out, res
    return out


# revision 13
# speedup vs baseline: 1.1858x; 1.1858x over previous
import sys
from contextlib import ExitStack

sys.path.insert(0, "/opt/trn_rl_repo")

import numpy as np
import ml_dtypes

import concourse.bass as bass
import concourse.bacc as bacc
import concourse.mybir as mybir
import concourse.tile as tile
from concourse.bass_utils import run_bass_kernel_spmd
from concourse.masks import make_identity

B, N, D, H = 4, 4096, 1024, 16
HD = D // H
NCORES = 8
T = (B * N) // NCORES  # 2048 tokens per core
P = 128
NT = T // P            # 16 token tiles per core
KT = D // P            # 8 contraction tiles
E3 = 3 * D

GP_ATTN = 0   # gpsimd offload disabled: SBUF-BW contention nets zero

_CACHE = {}


def _name(t):
    return t.name if hasattr(t, "name") else t.tensor.name


def _build():
    bf = mybir.dt.bfloat16
    f32 = mybir.dt.float32
    X = mybir.AxisListType.X
    nc = bacc.Bacc(None, target_bir_lowering=False)
    names = {}
    with tile.TileContext(nc) as tc:
        with ExitStack() as ctx:
            dram = ctx.enter_context(tc.tile_pool(name="dram", bufs=1, space="DRAM"))
            xT_d = dram.tile([D, T], bf, kind="ExternalInput")
            wq_d = dram.tile([D, E3], bf, kind="ExternalInput")
            wo_d = dram.tile([D, D], bf, kind="ExternalInput")
            out_d = dram.tile([T, D], f32, kind="ExternalOutput")
            names["xT"] = _name(xT_d)
            names["wqkvT"] = _name(wq_d)
            names["woT"] = _name(wo_d)
            names["out"] = _name(out_d)

            consts = ctx.enter_context(tc.tile_pool(name="consts", bufs=1))
            xT_sb = consts.tile([P, KT, T], bf)
            wq_sb = consts.tile([P, KT, E3], bf)
            wo_sb = consts.tile([P, KT, D], bf)
            ident = consts.tile([P, P], bf)
            make_identity(nc, ident)
            # split input DMAs so tile-0 compute starts as soon as its
            # slices land (whole-tensor uploads cost ~47us of head idle)
            nc.sync.dma_start(
                out=xT_sb[:, :, 0:P],
                in_=xT_d[:, 0:P].rearrange("(k p) t -> p k t", p=P),
            )
            for nch in range(E3 // 512):
                esl = bass.ts(nch, 512)
                nc.sync.dma_start(
                    out=wq_sb[:, :, esl],
                    in_=wq_d[:, esl].rearrange("(k p) e -> p k e", p=P),
                )
            for i in range(1, NT):
                tsl = bass.ts(i, P)
                nc.sync.dma_start(
                    out=xT_sb[:, :, tsl],
                    in_=xT_d[:, tsl].rearrange("(k p) t -> p k t", p=P),
                )
            nc.sync.dma_start(out=wo_sb[:], in_=wo_d[:].rearrange("(k p) e -> p k e", p=P))

            pool = ctx.enter_context(tc.tile_pool(name="work", bufs=2))
            qpool = ctx.enter_context(tc.tile_pool(name="qkvp", bufs=3))
            scratch = ctx.enter_context(tc.tile_pool(name="scratch", bufs=1))
            psum1 = ctx.enter_context(tc.tile_pool(name="psum1", bufs=2, space="PSUM"))
            psum2 = ctx.enter_context(tc.tile_pool(name="psum2", bufs=2, space="PSUM"))
            psum3 = ctx.enter_context(tc.tile_pool(name="psum3", bufs=2, space="PSUM"))

            # warm the PE HAM clock-gate while input DMAs stream.
            # Must be real MATMULs: transpose-mode does not count as PE-busy
            # for HAM, so a transpose warmup leaves the first QKV tile cold.
            for _ in range(32):
                wp = psum2.tile([P, P], f32, tag="pt_warm")
                nc.tensor.matmul(wp[:], ident[:], ident[:], start=True, stop=True)

            # stage1(i): qkv proj + score muls + reduces + softmax -> wn
            # stage2(i): attn (DVE+GP) + merge + transpose + out proj
            # Emitted as s1(0), s1(1), s2(0), s1(2), s2(1), ... so DVE always
            # has stage-1 work queued while GPSIMD chews on stage-2 groups.
            stage2_state = {}

            qkv_state = {}

            def stage1a(i):
                tsl = bass.ts(i, P)
                qkv = qpool.tile([P, E3], bf, tag="qkv")
                for nch in range(E3 // 512):
                    ps = psum1.tile([P, 512], f32, tag="mm1")
                    for k in range(KT):
                        nc.tensor.matmul(
                            ps[:],
                            xT_sb[:, k, tsl],
                            wq_sb[:, k, bass.ts(nch, 512)],
                            start=(k == 0),
                            stop=(k == KT - 1),
                        )
                    nc.scalar.copy(qkv[:, bass.ts(nch, 512)], ps[:])
                qkv_state[i] = qkv

            def stage1b(i):
                qkv = qkv_state[i]
                qv3 = qkv[:, 0:D].rearrange("p (h d) -> p h d", d=HD)
                kv3 = qkv[:, D : 2 * D].rearrange("p (g d) -> p g d", d=HD)
                # one 2x-mode mul for all (h,g) products, then 2x pairwise tree
                big = scratch.tile([P, H * H * HD], bf, tag="big")
                t1 = scratch.tile([P, 8192], bf, tag="t1")
                t2 = scratch.tile([P, 4096], bf, tag="t2")
                t3 = scratch.tile([P, 2048], bf, tag="t3")
                t4 = scratch.tile([P, 1024], bf, tag="t4")
                t5 = scratch.tile([P, 512], bf, tag="t5")
                bigS = big[:].rearrange("p (h g d) -> p h g d", h=H, g=H)
                nc.vector.tensor_mul(
                    bigS,
                    qv3[:, :, None, :].broadcast_to((P, H, H, HD)),
                    kv3[:, None, :, :].broadcast_to((P, H, H, HD)),
                )
                T1 = t1[:].rearrange("p (h g d) -> p h g d", h=H, g=H)
                T2 = t2[:].rearrange("p (h g d) -> p h g d", h=H, g=H)
                T3 = t3[:].rearrange("p (h g d) -> p h g d", h=H, g=H)
                T4 = t4[:].rearrange("p (h g d) -> p h g d", h=H, g=H)
                T5 = t5[:].rearrange("p (h g d) -> p h g d", h=H, g=H)
                nc.vector.tensor_add(T1, bigS[:, :, :, 0:32], bigS[:, :, :, 32:64])
                nc.vector.tensor_add(T2, T1[:, :, :, 0:16], T1[:, :, :, 16:32])
                nc.vector.tensor_add(T3, T2[:, :, :, 0:8], T2[:, :, :, 8:16])
                nc.vector.tensor_add(T4, T3[:, :, :, 0:4], T3[:, :, :, 4:8])
                nc.vector.tensor_add(T5, T4[:, :, :, 0:2], T4[:, :, :, 2:4])
                scores = pool.tile([P, H, H], f32, tag="sc")  # [p, h, g]
                nc.vector.tensor_add(
                    scores[:, :, :, None], T5[:, :, :, 0:1], T5[:, :, :, 1:2]
                )

                # softmax over g (innermost of [p, h, g]); no max-subtract
                we = pool.tile([P, H, H], bf, tag="we")  # exp scores [p, h, g]
                den = pool.tile([P, H], f32, tag="den")
                rec = pool.tile([P, H], f32, tag="rec")
                wn = pool.tile([P, H, H], bf, tag="wn")  # normalized w [p, h, g]
                nc.scalar.activation(
                    we[:], scores[:], mybir.ActivationFunctionType.Exp, scale=1.0 / 32.0
                )
                stage2_state[i] = (we, den, rec, wn, big, t1, t2, t3)

            def stage2(i):
                tsl = bass.ts(i, P)
                we, den, rec, wn, big, t1, t2, t3 = stage2_state.pop(i)
                qkv = qkv_state.pop(i)
                nc.vector.reduce_sum(den[:, :, None], we[:], axis=X)
                nc.vector.reciprocal(rec[:], den[:])
                recb = rec[:, :, None].broadcast_to((P, H, H))
                nc.vector.tensor_mul(wn[:], we[:], recb)
                # attn[p, h, d] = sum_g wn[p, h, g] * v[p, d, g]
                # (host pre-permuted the V weight block so v lands as [d, g])
                vv = qkv[:, 2 * D : 3 * D].rearrange("p (d g) -> p d g", g=H)
                bigA = big[:].rearrange("p (h d g) -> p h d g", h=H, d=HD)
                nc.vector.tensor_mul(
                    bigA,
                    wn[:, :, None, :].broadcast_to((P, H, HD, H)),
                    vv[:, None, :, :].broadcast_to((P, H, HD, H)),
                )
                A1 = t1[:].rearrange("p (h d g) -> p h d g", h=H, d=HD)
                A2 = t2[:].rearrange("p (h d g) -> p h d g", h=H, d=HD)
                A3 = t3[:].rearrange("p (h d g) -> p h d g", h=H, d=HD)
                attn_dve = pool.tile([P, H, HD], bf, tag="attn_dve")
                nc.vector.tensor_add(A1, bigA[:, :, :, 0:8], bigA[:, :, :, 8:16])
                nc.vector.tensor_add(A2, A1[:, :, :, 0:4], A1[:, :, :, 4:8])
                nc.vector.tensor_add(A3, A2[:, :, :, 0:2], A2[:, :, :, 2:4])
                nc.vector.tensor_add(
                    attn_dve[:, :, :, None], A3[:, :, :, 0:1], A3[:, :, :, 1:2]
                )

                # transpose attn tile -> [e, t] blocks for output proj
                attnb_flat = attn_dve.rearrange("p h d -> p (h d)")
                attnT = pool.tile([P, KT, P], bf, tag="attnT")
                for c in range(KT):
                    pt = psum2.tile([P, P], bf, tag="pt")
                    nc.tensor.transpose(pt[:], attnb_flat[:, bass.ts(c, P)], ident[:])
                    nc.scalar.copy(attnT[:, c, :], pt[:])

                outt = pool.tile([P, D], f32, tag="outt")
                for nch in range(D // 512):
                    po = psum3.tile([P, 512], f32, tag="po")
                    for k in range(KT):
                        nc.tensor.matmul(
                            po[:],
                            attnT[:, k, :],
                            wo_sb[:, k, bass.ts(nch, 512)],
                            start=(k == 0),
                            stop=(k == KT - 1),
                        )
                    nc.scalar.copy(outt[:, bass.ts(nch, 512)], po[:])
                nc.sync.dma_start(out=out_d[tsl, :], in_=outt[:])

            stage1a(0)
            stage1a(1)
            stage1b(0)
            for i in range(0, NT - 2):
                stage1a(i + 2)
                stage1b(i + 1)
                stage2(i)
            stage1b(NT - 1)
            stage2(NT - 2)
            stage2(NT - 1)
    nc.compile()
    return nc, names


def kernel(x, Wqkv, Wo, bo, trace=False):
    if "nc" not in _CACHE:
        _CACHE["nc"], _CACHE["names"] = _build()
    nc, names = _CACHE["nc"], _CACHE["names"]
    bf = ml_dtypes.bfloat16
    xt = np.ascontiguousarray(
        np.asarray(x, dtype=np.float32).reshape(B * N, D).T
    )  # [D, B*N]
    wqkvT_f = np.ascontiguousarray(np.asarray(Wqkv, dtype=np.float32).T)  # [D, 3D]
    vblk = wqkvT_f[:, 2 * D :].reshape(D, H, HD)
    wqkvT_f[:, 2 * D :] = np.ascontiguousarray(vblk.transpose(0, 2, 1)).reshape(D, D)
    wqkvT = wqkvT_f.astype(bf)
    woT = np.ascontiguousarray(np.asarray(Wo, dtype=np.float32).T).astype(bf)
    in_maps = []
    for c in range(NCORES):
        shard = np.ascontiguousarray(xt[:, c * T : (c + 1) * T]).astype(bf)
        in_maps.append(
            {names["xT"]: shard, names["wqkvT"]: wqkvT, names["woT"]: woT}
        )
    res = run_bass_kernel_spmd(
        nc, in_maps, core_ids=list(range(NCORES)), trace=trace
    )
    shards = [res.results[c][names["out"]] for c in range(NCORES)]
    out = np.concatenate(shards, axis=0).reshape(B, N, D).astype(np.float32)
    out = out + np.asarray(bo, dtype=np.float32)[None, None, :]
    if trace:
        return out, res
    return out
